# revision 9
# baseline (speedup 1.0000x reference)
"""DistanceFromAnswerLoss on 8 Trainium2 NeuronCores — v2.

out = 0.1 * sum_{b,c} mask[b,c] * exp(input[b,c])
  mask[b,c] = |c - t_b| / sqrt(sum_c (c - t_b)^2),  mask = 0 where t_b == 0

Data-parallel: rows sorted by t on the host, 512/core, shipped transposed
(columns on partitions) in bf16.  Per core a contiguous 16-block window
(2048 columns) covers every t; outside it sign(c - t_b) is constant per
block, so with m = window center:

  sum_{c in agg} |c-t_b| e[c,b] = A~[b] + (m - t_b) * S~[b]
    A~ = sum +-(c-m) e,  S~ = sum +-e     (per-block 2-col matmuls)

For the 16 window blocks the weight is folded into exp's argument on the
host (log-trick):  |c-t| e^x = e^{x + ln|c-t|}, so the window payload is
bf16(x + ln|c-t|) and its contribution W~[b] = sum_win e rides the A~ PSUM
row via a [1, 0] stationary column (both rows are finally dotted with the
same per-row scales sct / sc2 = scale, scale*(m-t)).

exp runs entirely on the DVE as a Schraudolph bf16-bitcast:
  e^x ~= bitcast_bf16(int16(floor(x * 128/ln2 + (127*128 - CADJ))))
one 4x-mode tensor_scalar per tile (~1.1us/[128,4096]), leaving ACT idle
and the kernel DMA-bound (~8.4 MB bf16 at ~390 GB/s).  CADJ calibrated
so the sawtooth's geometric mean is 1 (floor semantics).

Device per core: 8 tile DMAs -> 8 DVE tensor_scalar -> 64 matmuls into
one [2,512] PSUM group -> 2 DVE dot products -> out [1,2].
"""

import sys
from contextlib import ExitStack

import numpy as np
import ml_dtypes

sys.path.insert(0, "/opt/trn_rl_repo")

import concourse.bass as bass
import concourse.tile as tile
from concourse import bacc, mybir
from concourse.bass_utils import run_bass_kernel_spmd

B = 4096
C = 8192
N_CORES = 8
ROWS = B // N_CORES          # 512 rows (free dim) per core
NQ = C // 128                # 64 column blocks of 128 (partition dim)
NS = 16                      # window blocks (log-baked weights, contiguous)
NAGG = NQ - NS               # 48 aggregate blocks
W = 4096                     # x tile width (8 slots of ROWS)
NT = (NQ * ROWS) // W        # 8 tiles of [128, 4096] per core
SLOTS = W // ROWS            # 8 slots per tile
COEFF = 0.1

SCHR_SCALE = float(np.float32(128.0 / np.log(2.0)))
CADJ = 7.33                  # sawtooth centering (HW convert rounds-to-nearest)
SCHR_BIAS = float(np.float32(127.0 * 128.0 - CADJ))
LW_CLAMP = -50.0             # ln-weight clamp (e^{x-50} ~ 0)

F32 = mybir.dt.float32
BF16 = mybir.dt.bfloat16
I16 = mybir.dt.int16
Op = mybir.AluOpType


def _build() -> bass.Bass:
    nc = bacc.Bacc("TRN2", target_bir_lowering=False, debug=False)
    x = nc.declare_dram_parameter("x", [128, NQ * ROWS], BF16, isOutput=False)
    wv = nc.declare_dram_parameter("wv", [128, 2 * NQ], BF16, isOutput=False)
    scs = nc.declare_dram_parameter("scs", [2, ROWS], F32, isOutput=False)
    out = nc.declare_dram_parameter("out", [2, 2], F32, isOutput=True)

    # x stream chunks (in slots of ROWS columns): head/tail split in half
    # for faster pipeline fill + a short tail; DMAs alternate between the
    # sync and gpsimd HWDGE rings.
    CHUNKS = [4, 4, 8, 8, 8, 8, 8, 8, 4, 4]
    QCUT = NQ - SLOTS            # slots 0..55 -> PSUM group A, 56..63 -> B

    with tile.TileContext(nc) as tc, ExitStack() as ctx:
        const_pool = ctx.enter_context(tc.tile_pool(name="const", bufs=1))
        xpool = ctx.enter_context(tc.tile_pool(name="x", bufs=1))
        epool = ctx.enter_context(tc.tile_pool(name="e", bufs=1))
        spool = ctx.enter_context(tc.tile_pool(name="s", bufs=1))
        psum_pool = ctx.enter_context(tc.tile_pool(name="ps", bufs=1, space="PSUM"))

        # constants ride the scalar HWDGE ring (lands ahead of the x stream)
        wvt = const_pool.tile([128, 2 * NQ], BF16)
        nc.scalar.dma_start(out=wvt[:], in_=wv[:, :])
        scst = const_pool.tile([2, ROWS], F32)
        nc.scalar.dma_start(out=scst[:], in_=scs[:, :])

        xt = []
        q0 = 0
        for j, nsl in enumerate(CHUNKS):
            t = xpool.tile([128, nsl * ROWS], BF16, name=f"x{j}")
            eng = nc.sync if j % 2 == 0 else nc.gpsimd
            eng.dma_start(out=t[:], in_=x[:, q0 * ROWS:(q0 + nsl) * ROWS])
            xt.append(t)
            q0 += nsl

        aspsA = psum_pool.tile([2, ROWS], F32, tag="pasA")
        aspsB = psum_pool.tile([2, ROWS], F32, tag="pasB")
        res = spool.tile([2, 2], F32)

        q0 = 0
        for j, nsl in enumerate(CHUNKS):
            et = epool.tile([128, nsl * ROWS], I16, name=f"e{j}")
            nc.vector.tensor_scalar(
                et[:], xt[j][:], SCHR_SCALE, SCHR_BIAS, op0=Op.mult, op1=Op.add
            )
            eb = et[:].bitcast(BF16)
            for k in range(nsl):
                q = q0 + k
                ps = aspsA if q < QCUT else aspsB
                nc.tensor.matmul(
                    ps[:], wvt[:, 2 * q:2 * q + 2],
                    eb[:, k * ROWS:(k + 1) * ROWS],
                    start=(q == 0 or q == QCUT),
                    stop=(q == QCUT - 1 or q == NQ - 1),
                )
                if q == QCUT - 1:
                    # group A combine runs mid-stream, hidden under the tail
                    jA = spool.tile([2, ROWS], F32, name="jA")
                    nc.vector.scalar_tensor_tensor(
                        jA[:], aspsA[:], 0.0, scst[:], op0=Op.add, op1=Op.mult,
                        accum_out=res[:, 0:1],
                    )
            q0 += nsl

        jB = spool.tile([2, ROWS], F32, name="jB")
        nc.vector.scalar_tensor_tensor(
            jB[:], aspsB[:], 0.0, scst[:], op0=Op.add, op1=Op.mult,
            accum_out=res[:, 1:2],
        )
        nc.scalar.dma_start(out=out[:, :], in_=res[:])

    nc.finalize()
    return nc


_NC = None


def _get_nc() -> bass.Bass:
    global _NC
    if _NC is None:
        _NC = _build()
    return _NC


def _plan(target: np.ndarray):
    """Sort rows by target; per core pick a contiguous 16-block window
    covering all its targets and a block permutation [agg | window]."""
    t = np.asarray(target).astype(np.int64).reshape(B)
    order = np.argsort(t, kind="stable")
    plans = []
    for k in range(N_CORES):
        rows = order[k * ROWS:(k + 1) * ROWS]
        tc = t[rows]
        blo, bhi = int(tc.min()) >> 7, int(tc.max()) >> 7
        span = bhi - blo + 1
        assert span <= NS, f"target spread too wide for window: {span} blocks"
        wlo = min(max(blo - (NS - span) // 2, 0), NQ - NS)
        assert wlo <= blo and bhi < wlo + NS
        win = np.arange(wlo, wlo + NS)
        rest = np.array([q for q in range(NQ) if q < wlo or q >= wlo + NS])
        plans.append((rows, tc, win, rest))
    return plans


def make_in_maps(input: np.ndarray, target: np.ndarray) -> list[dict]:
    xf = np.asarray(input, dtype=np.float32)
    plans = _plan(target)
    # row norm (exact closed form): sum_c (c-t)^2 = C*t^2 - 2*t*S1 + S2
    s1 = (C - 1) * C // 2
    s2 = (C - 1) * C * (2 * C - 1) // 6
    in_maps = []
    p128 = np.arange(128, dtype=np.float64)
    for rows, tc, win, rest in plans:
        # slot order: 48 agg blocks then 16 window blocks
        perm = np.concatenate([rest, win])
        m = float(win[0] * 128 + (NS * 128) / 2.0)
        # payload: agg slots = x; window slots = x + ln|c - t| (clamped)
        xk = xf[rows].reshape(ROWS, NQ, 128)[:, perm, :]  # [ROWS, slot, 128]
        xT = np.ascontiguousarray(xk.transpose(2, 1, 0))  # [128, slot, ROWS]
        cw = (win[None, :] * 128 + p128[:, None])          # [128, NS] c values
        dist = np.abs(cw[:, :, None] - tc[None, None, :].astype(np.float64))
        lw = np.log(np.maximum(dist, 1e-30))
        np.maximum(lw, LW_CLAMP, out=lw)
        xT[:, NAGG:, :] += lw.astype(np.float32)
        xb = xT.reshape(128, NQ * ROWS).astype(ml_dtypes.bfloat16)

        # stationary columns: agg [-+(c-m), -+1]; window [1, 0]
        sgn = np.where(rest * 128 > win[-1] * 128, 1.0, -1.0)
        cagg = rest[None, :] * 128 + p128[:, None] - m      # [128, NAGG]
        wvc = np.zeros((128, 2 * NQ), dtype=np.float32)
        wvc[:, 0:2 * NAGG:2] = cagg * sgn[None, :]
        wvc[:, 1:2 * NAGG:2] = sgn[None, :]
        wvc[:, 2 * NAGG::2] = 1.0

        norm = np.sqrt(C * tc.astype(np.float64) ** 2 - 2.0 * tc * s1 + s2)
        sc64 = COEFF / np.maximum(norm, 1e-12) * (tc != 0)
        scs = np.stack([sc64, sc64 * (m - tc.astype(np.float64))])
        in_maps.append({
            "x": xb,
            "wv": wvc.astype(ml_dtypes.bfloat16),
            "scs": np.ascontiguousarray(scs.astype(np.float32)),
        })
    return in_maps


def run(input: np.ndarray, target: np.ndarray, trace: bool = False, tmpdir=None):
    nc = _get_nc()
    in_maps = make_in_maps(input, target)
    res = run_bass_kernel_spmd(
        nc, in_maps, list(range(N_CORES)), trace=trace, tmpdir=tmpdir
    )
    total = np.float32(0.0)
    for r in res.results:
        total += np.float32(r["out"].reshape(-1).sum())
    return np.asarray(total, dtype=np.float32), res


def kernel(input: np.ndarray, target: np.ndarray) -> np.ndarray:
    out, _ = run(input, target)
    return out


# revision 11
# speedup vs baseline: 1.0752x; 1.0752x over previous
"""DistanceFromAnswerLoss on 8 Trainium2 NeuronCores — v2.

out = 0.1 * sum_{b,c} mask[b,c] * exp(input[b,c])
  mask[b,c] = |c - t_b| / sqrt(sum_c (c - t_b)^2),  mask = 0 where t_b == 0

Data-parallel: rows sorted by t on the host, 512/core, shipped transposed
(columns on partitions) in bf16.  Per core a contiguous 16-block window
(2048 columns) covers every t; outside it sign(c - t_b) is constant per
block, so with m = window center:

  sum_{c in agg} |c-t_b| e[c,b] = A~[b] + (m - t_b) * S~[b]
    A~ = sum +-(c-m) e,  S~ = sum +-e     (per-block 2-col matmuls)

For the 16 window blocks the weight is folded into exp's argument on the
host (log-trick):  |c-t| e^x = e^{x + ln|c-t|}, so the window payload is
bf16(x + ln|c-t|) and its contribution W~[b] = sum_win e rides the A~ PSUM
row via a [1, 0] stationary column (both rows are finally dotted with the
same per-row scales sct / sc2 = scale, scale*(m-t)).

exp runs entirely on the DVE as a Schraudolph bf16-bitcast:
  e^x ~= bitcast_bf16(int16(floor(x * 128/ln2 + (127*128 - CADJ))))
one 4x-mode tensor_scalar per tile (~1.1us/[128,4096]), leaving ACT idle
and the kernel DMA-bound (~8.4 MB bf16 at ~390 GB/s).  CADJ calibrated
so the sawtooth's geometric mean is 1 (floor semantics).

Device per core: 8 tile DMAs -> 8 DVE tensor_scalar -> 64 matmuls into
one [2,512] PSUM group -> 2 DVE dot products -> out [1,2].
"""

import sys
from contextlib import ExitStack

import numpy as np
import ml_dtypes

sys.path.insert(0, "/opt/trn_rl_repo")

import concourse.bass as bass
import concourse.tile as tile
from concourse import bacc, mybir
from concourse.bass_utils import run_bass_kernel_spmd

B = 4096
C = 8192
N_CORES = 8
ROWS = B // N_CORES          # 512 rows (free dim) per core
NQ = C // 128                # 64 column blocks of 128 (partition dim)
NS = 16                      # window blocks (log-baked weights, contiguous)
NAGG = NQ - NS               # 48 aggregate blocks
W = 4096                     # x tile width (8 slots of ROWS)
NT = (NQ * ROWS) // W        # 8 tiles of [128, 4096] per core
SLOTS = W // ROWS            # 8 slots per tile
COEFF = 0.1

SCHR_SCALE = float(np.float32(128.0 / np.log(2.0)))
CADJ = 7.33                  # sawtooth centering (HW convert rounds-to-nearest)
SCHR_BIAS = float(np.float32(127.0 * 128.0 - CADJ))
LW_CLAMP = -50.0             # ln-weight clamp (e^{x-50} ~ 0)

F32 = mybir.dt.float32
BF16 = mybir.dt.bfloat16
I16 = mybir.dt.int16
Op = mybir.AluOpType


def _build() -> bass.Bass:
    nc = bacc.Bacc("TRN2", target_bir_lowering=False, debug=False)
    x = nc.declare_dram_parameter("x", [128, NQ * ROWS], BF16, isOutput=False)
    wv = nc.declare_dram_parameter("wv", [128, 2 * NQ], BF16, isOutput=False)
    scs = nc.declare_dram_parameter("scs", [2, ROWS], F32, isOutput=False)
    out = nc.declare_dram_parameter("out", [2, 2], F32, isOutput=True)

    # x stream chunks (in slots of ROWS columns): head/tail split in half
    # for faster pipeline fill + a short tail; DMAs alternate between the
    # sync and gpsimd HWDGE rings.
    CHUNKS = [4, 4, 8, 8, 8, 8, 8, 8, 4, 4]
    QCUT = NQ - SLOTS            # slots 0..55 -> PSUM group A, 56..63 -> B

    with tile.TileContext(nc) as tc, ExitStack() as ctx:
        const_pool = ctx.enter_context(tc.tile_pool(name="const", bufs=1))
        xpool = ctx.enter_context(tc.tile_pool(name="x", bufs=1))
        epool = ctx.enter_context(tc.tile_pool(name="e", bufs=1))
        spool = ctx.enter_context(tc.tile_pool(name="s", bufs=1))
        psum_pool = ctx.enter_context(tc.tile_pool(name="ps", bufs=1, space="PSUM"))

        # constants ride the scalar HWDGE ring (lands ahead of the x stream)
        wvt = const_pool.tile([128, 2 * NQ], BF16)
        nc.scalar.dma_start(out=wvt[:], in_=wv[:, :])
        scst = const_pool.tile([2, ROWS], F32)
        nc.scalar.dma_start(out=scst[:], in_=scs[:, :])

        xt = []
        q0 = 0
        for j, nsl in enumerate(CHUNKS):
            t = xpool.tile([128, nsl * ROWS], BF16, name=f"x{j}")
            nc.sync.dma_start(out=t[:], in_=x[:, q0 * ROWS:(q0 + nsl) * ROWS])
            xt.append(t)
            q0 += nsl

        aspsA = psum_pool.tile([2, ROWS], F32, tag="pasA")
        aspsB = psum_pool.tile([2, ROWS], F32, tag="pasB")
        res = spool.tile([2, 2], F32)

        q0 = 0
        for j, nsl in enumerate(CHUNKS):
            et = epool.tile([128, nsl * ROWS], I16, name=f"e{j}")
            nc.vector.tensor_scalar(
                et[:], xt[j][:], SCHR_SCALE, SCHR_BIAS, op0=Op.mult, op1=Op.add
            )
            eb = et[:].bitcast(BF16)
            for k in range(nsl):
                q = q0 + k
                ps = aspsA if q < QCUT else aspsB
                nc.tensor.matmul(
                    ps[:], wvt[:, 2 * q:2 * q + 2],
                    eb[:, k * ROWS:(k + 1) * ROWS],
                    start=(q == 0 or q == QCUT),
                    stop=(q == QCUT - 1 or q == NQ - 1),
                )
                if q == QCUT - 1:
                    # group A combine runs mid-stream, hidden under the tail
                    jA = spool.tile([2, ROWS], F32, name="jA")
                    nc.vector.scalar_tensor_tensor(
                        jA[:], aspsA[:], 0.0, scst[:], op0=Op.add, op1=Op.mult,
                        accum_out=res[:, 0:1],
                    )
            q0 += nsl

        jB = spool.tile([2, ROWS], F32, name="jB")
        nc.vector.scalar_tensor_tensor(
            jB[:], aspsB[:], 0.0, scst[:], op0=Op.add, op1=Op.mult,
            accum_out=res[:, 1:2],
        )
        nc.sync.dma_start(out=out[:, :], in_=res[:])

    nc.finalize()
    return nc


_NC = None


def _get_nc() -> bass.Bass:
    global _NC
    if _NC is None:
        _NC = _build()
    return _NC


def _plan(target: np.ndarray):
    """Sort rows by target; per core pick a contiguous 16-block window
    covering all its targets and a block permutation [agg | window]."""
    t = np.asarray(target).astype(np.int64).reshape(B)
    order = np.argsort(t, kind="stable")
    plans = []
    for k in range(N_CORES):
        rows = order[k * ROWS:(k + 1) * ROWS]
        tc = t[rows]
        blo, bhi = int(tc.min()) >> 7, int(tc.max()) >> 7
        span = bhi - blo + 1
        assert span <= NS, f"target spread too wide for window: {span} blocks"
        wlo = min(max(blo - (NS - span) // 2, 0), NQ - NS)
        assert wlo <= blo and bhi < wlo + NS
        win = np.arange(wlo, wlo + NS)
        rest = np.array([q for q in range(NQ) if q < wlo or q >= wlo + NS])
        plans.append((rows, tc, win, rest))
    return plans


def make_in_maps(input: np.ndarray, target: np.ndarray) -> list[dict]:
    xf = np.asarray(input, dtype=np.float32)
    plans = _plan(target)
    # row norm (exact closed form): sum_c (c-t)^2 = C*t^2 - 2*t*S1 + S2
    s1 = (C - 1) * C // 2
    s2 = (C - 1) * C * (2 * C - 1) // 6
    in_maps = []
    p128 = np.arange(128, dtype=np.float64)
    for rows, tc, win, rest in plans:
        # slot order: 48 agg blocks then 16 window blocks
        perm = np.concatenate([rest, win])
        m = float(win[0] * 128 + (NS * 128) / 2.0)
        # payload: agg slots = x; window slots = x + ln|c - t| (clamped)
        xk = xf[rows].reshape(ROWS, NQ, 128)[:, perm, :]  # [ROWS, slot, 128]
        xT = np.ascontiguousarray(xk.transpose(2, 1, 0))  # [128, slot, ROWS]
        cw = (win[None, :] * 128 + p128[:, None])          # [128, NS] c values
        dist = np.abs(cw[:, :, None] - tc[None, None, :].astype(np.float64))
        lw = np.log(np.maximum(dist, 1e-30))
        np.maximum(lw, LW_CLAMP, out=lw)
        xT[:, NAGG:, :] += lw.astype(np.float32)
        xb = xT.reshape(128, NQ * ROWS).astype(ml_dtypes.bfloat16)

        # stationary columns: agg [-+(c-m), -+1]; window [1, 0]
        sgn = np.where(rest * 128 > win[-1] * 128, 1.0, -1.0)
        cagg = rest[None, :] * 128 + p128[:, None] - m      # [128, NAGG]
        wvc = np.zeros((128, 2 * NQ), dtype=np.float32)
        wvc[:, 0:2 * NAGG:2] = cagg * sgn[None, :]
        wvc[:, 1:2 * NAGG:2] = sgn[None, :]
        wvc[:, 2 * NAGG::2] = 1.0

        norm = np.sqrt(C * tc.astype(np.float64) ** 2 - 2.0 * tc * s1 + s2)
        sc64 = COEFF / np.maximum(norm, 1e-12) * (tc != 0)
        scs = np.stack([sc64, sc64 * (m - tc.astype(np.float64))])
        in_maps.append({
            "x": xb,
            "wv": wvc.astype(ml_dtypes.bfloat16),
            "scs": np.ascontiguousarray(scs.astype(np.float32)),
        })
    return in_maps


def run(input: np.ndarray, target: np.ndarray, trace: bool = False, tmpdir=None):
    nc = _get_nc()
    in_maps = make_in_maps(input, target)
    res = run_bass_kernel_spmd(
        nc, in_maps, list(range(N_CORES)), trace=trace, tmpdir=tmpdir
    )
    total = np.float32(0.0)
    for r in res.results:
        total += np.float32(r["out"].reshape(-1).sum())
    return np.asarray(total, dtype=np.float32), res


def kernel(input: np.ndarray, target: np.ndarray) -> np.ndarray:
    out, _ = run(input, target)
    return out


# revision 13
# speedup vs baseline: 1.2019x; 1.1178x over previous
"""DistanceFromAnswerLoss on 8 Trainium2 NeuronCores — v3 (fp8/bf16 split).

out = 0.1 * sum_{b,c} mask[b,c] * exp(input[b,c])
  mask[b,c] = |c - t_b| / sqrt(sum_c (c - t_b)^2),  mask = 0 where t_b == 0

Host: rows sorted by t, 512/core, transposed (columns on partitions).
Per core a contiguous 16-block window covers every t; outside it
sign(c - t_b) is constant per 128-column block, so with m = window center

  sum_{c in agg} |c-t_b| e[c,b] = A~[b] + (m - t_b) * S~[b]
    A~ = sum +-(c-m) e,  S~ = sum +-e     (per-block 2-col matmuls)

Window blocks: weights are folded into exp's argument on the host
(|c-t| e^x = e^{x + ln|c-t|}); their sum rides the A~ PSUM row via a
[1, 0] stationary column.  Final: rows dotted with scale / scale*(m-t).

exp is split across two engines:
 - 32 aggregate blocks ship as fp8_e4m3 and run on ACT (dtype-blind
   1 elem/lane/cycle); an early dummy exp pulls the ~2.7us table load
   into the DMA spin-up dead time;
 - 16 aggregate + 16 window blocks ship as bf16 and run on the DVE as a
   Schraudolph bitcast exp (one 4x-mode tensor_scalar per chunk:
   e^x ~= bitcast_bf16(int16(x * 128/ln2 + (127*128 - CADJ)))).

The x stream is 12 uniform [128 x 4KB] transfers (fp8 chunk = 8 slots,
bf16 chunk = 4 slots; 6.29 MB total) on the single sync HWDGE ring —
mixed-size chunks provably unbalance the per-engine DMA queues and grow
a multi-us ragged tail.  ACT chunks are interleaved so the ACT chain
(longest engine, ~15us) never starves; window chunks stream last, and
the PSUM accumulation is split in two groups so all but the final 4
slots combine mid-stream.
"""

import sys
from contextlib import ExitStack

import numpy as np
import ml_dtypes

sys.path.insert(0, "/opt/trn_rl_repo")

import concourse.bass as bass
import concourse.tile as tile
from concourse import bacc, mybir
from concourse.bass_utils import run_bass_kernel_spmd

B = 4096
C = 8192
N_CORES = 8
ROWS = B // N_CORES          # 512 rows (free dim) per core
NQ = C // 128                # 64 column blocks of 128 (partition dim)
NS = 16                      # window blocks (log-baked weights, contiguous)
NAGG = NQ - NS               # 48 aggregate blocks
COEFF = 0.1

SCHR_SCALE = float(np.float32(128.0 / np.log(2.0)))
CADJ = 7.33                  # sawtooth centering (HW convert rounds)
SCHR_BIAS = float(np.float32(127.0 * 128.0 - CADJ))
LW_CLAMP = -50.0             # ln-weight clamp (e^{x-50} ~ 0)

# stream plan: A = 8 fp8 agg slots on ACT, D = 4 bf16 slots on DVE.
# Every chunk is a [128, 4KB] transfer.  A-chunks take agg blocks
# rest[0:32]; D-chunks take rest[32:48] then the 16 window blocks.
CHUNK_PLAN = ["A", "A", "D", "A", "D", "D", "A", "D", "D", "D", "D", "D"]
NA = CHUNK_PLAN.count("A")           # 4 fp8 chunks
ND = CHUNK_PLAN.count("D")           # 8 bf16 chunks
QCUT = NQ - 4                        # last D chunk -> PSUM group B

F32 = mybir.dt.float32
BF16 = mybir.dt.bfloat16
FP8 = mybir.dt.float8e4
I16 = mybir.dt.int16
Op = mybir.AluOpType
Af = mybir.ActivationFunctionType


def _build() -> bass.Bass:
    nc = bacc.Bacc("TRN2", target_bir_lowering=False, debug=False)
    x8 = nc.declare_dram_parameter("x8", [128, NA * 8 * ROWS], FP8, isOutput=False)
    xb = nc.declare_dram_parameter("xb", [128, ND * 4 * ROWS], BF16, isOutput=False)
    wv = nc.declare_dram_parameter("wv", [128, 2 * NQ], BF16, isOutput=False)
    scs = nc.declare_dram_parameter("scs", [2, ROWS], F32, isOutput=False)
    out = nc.declare_dram_parameter("out", [2, 2], F32, isOutput=True)

    with tile.TileContext(nc) as tc, ExitStack() as ctx:
        const_pool = ctx.enter_context(tc.tile_pool(name="const", bufs=1))
        xpool = ctx.enter_context(tc.tile_pool(name="x", bufs=1))
        epool = ctx.enter_context(tc.tile_pool(name="e", bufs=1))
        spool = ctx.enter_context(tc.tile_pool(name="s", bufs=1))
        psum_pool = ctx.enter_context(tc.tile_pool(name="ps", bufs=1, space="PSUM"))

        # ACT table load happens during the DMA spin-up dead time
        warm = const_pool.tile([128, 1], BF16)
        nc.vector.memset(warm[:], 0.0)
        warme = const_pool.tile([128, 1], BF16)
        nc.scalar.activation(warme[:], warm[:], Af.Exp)

        # constants on the scalar HWDGE ring
        wvt = const_pool.tile([128, 2 * NQ], BF16)
        nc.scalar.dma_start(out=wvt[:], in_=wv[:, :])
        scst = const_pool.tile([2, ROWS], F32)
        nc.scalar.dma_start(out=scst[:], in_=scs[:, :])

        # x stream: uniform 4KB-per-partition chunks on the sync ring
        xt = []
        a0 = d0 = 0
        for j, typ in enumerate(CHUNK_PLAN):
            if typ == "A":
                t = xpool.tile([128, 8 * ROWS], FP8, name=f"xa{j}")
                nc.sync.dma_start(
                    out=t[:], in_=x8[:, a0 * ROWS:(a0 + 8) * ROWS]
                )
                a0 += 8
            else:
                t = xpool.tile([128, 4 * ROWS], BF16, name=f"xd{j}")
                nc.sync.dma_start(
                    out=t[:], in_=xb[:, d0 * ROWS:(d0 + 4) * ROWS]
                )
                d0 += 4
            xt.append(t)

        aspsA = psum_pool.tile([2, ROWS], F32, tag="pasA")
        aspsB = psum_pool.tile([2, ROWS], F32, tag="pasB")
        res = spool.tile([2, 2], F32)

        q = 0
        for j, typ in enumerate(CHUNK_PLAN):
            if typ == "A":
                nsl = 8
                et = epool.tile([128, nsl * ROWS], BF16, name=f"e{j}")
                nc.scalar.activation(et[:], xt[j][:], Af.Exp)
                eb = et[:]
            else:
                nsl = 4
                et = epool.tile([128, nsl * ROWS], I16, name=f"e{j}")
                nc.vector.tensor_scalar(
                    et[:], xt[j][:], SCHR_SCALE, SCHR_BIAS,
                    op0=Op.mult, op1=Op.add,
                )
                eb = et[:].bitcast(BF16)
            for k in range(nsl):
                ps = aspsA if q < QCUT else aspsB
                nc.tensor.matmul(
                    ps[:], wvt[:, 2 * q:2 * q + 2],
                    eb[:, k * ROWS:(k + 1) * ROWS],
                    start=(q == 0 or q == QCUT),
                    stop=(q == QCUT - 1 or q == NQ - 1),
                )
                if q == QCUT - 1:
                    # group A combine runs mid-stream, hidden under the tail
                    jA = spool.tile([2, ROWS], F32, name="jA")
                    nc.vector.scalar_tensor_tensor(
                        jA[:], aspsA[:], 0.0, scst[:],
                        op0=Op.add, op1=Op.mult, accum_out=res[:, 0:1],
                    )
                q += 1

        jB = spool.tile([2, ROWS], F32, name="jB")
        nc.vector.scalar_tensor_tensor(
            jB[:], aspsB[:], 0.0, scst[:], op0=Op.add, op1=Op.mult,
            accum_out=res[:, 1:2],
        )
        nc.sync.dma_start(out=out[:, :], in_=res[:])

    nc.finalize()
    return nc


_NC = None


def _get_nc() -> bass.Bass:
    global _NC
    if _NC is None:
        _NC = _build()
    return _NC


def _plan(target: np.ndarray):
    """Sort rows by target; per core pick a contiguous 16-block window
    covering all its targets and the agg/window block split."""
    t = np.asarray(target).astype(np.int64).reshape(B)
    order = np.argsort(t, kind="stable")
    plans = []
    for k in range(N_CORES):
        rows = order[k * ROWS:(k + 1) * ROWS]
        tc = t[rows]
        blo, bhi = int(tc.min()) >> 7, int(tc.max()) >> 7
        span = bhi - blo + 1
        assert span <= NS, f"target spread too wide for window: {span} blocks"
        wlo = min(max(blo - (NS - span) // 2, 0), NQ - NS)
        assert wlo <= blo and bhi < wlo + NS
        win = np.arange(wlo, wlo + NS)
        rest = np.array([q for q in range(NQ) if q < wlo or q >= wlo + NS])
        plans.append((rows, tc, win, rest))
    return plans


def _stream_slots(win, rest):
    """Block id per stream slot, and per-buffer block lists."""
    a_blocks = list(rest[:NA * 8])
    d_blocks = list(rest[NA * 8:]) + list(win)
    slots = []
    ai = di = 0
    for typ in CHUNK_PLAN:
        if typ == "A":
            slots += a_blocks[ai:ai + 8]
            ai += 8
        else:
            slots += d_blocks[di:di + 4]
            di += 4
    return np.array(slots), np.array(a_blocks), np.array(d_blocks)


def make_in_maps(input: np.ndarray, target: np.ndarray) -> list[dict]:
    xf = np.asarray(input, dtype=np.float32)
    plans = _plan(target)
    s1 = (C - 1) * C // 2
    s2 = (C - 1) * C * (2 * C - 1) // 6
    in_maps = []
    p128 = np.arange(128, dtype=np.float64)
    for rows, tc, win, rest in plans:
        slots, a_blocks, d_blocks = _stream_slots(win, rest)
        m = float(win[0] * 128 + (NS * 128) / 2.0)
        xr = xf[rows].reshape(ROWS, NQ, 128)

        # fp8 payload: 32 agg blocks in A-chunk order
        xa = xr[:, a_blocks, :]                      # [ROWS, 32, 128]
        x8 = np.ascontiguousarray(xa.transpose(2, 1, 0)).reshape(
            128, NA * 8 * ROWS
        ).astype(ml_dtypes.float8_e4m3)

        # bf16 payload: 16 agg blocks then 16 window blocks (D-chunk order)
        xd = xr[:, d_blocks, :].transpose(2, 1, 0).astype(np.float64)
        cw = win[None, :] * 128 + p128[:, None]       # [128, NS]
        dist = np.abs(cw[:, :, None] - tc[None, None, :].astype(np.float64))
        lw = np.log(np.maximum(dist, 1e-30))
        np.maximum(lw, LW_CLAMP, out=lw)
        xd[:, ND * 4 - NS:, :] += lw
        xbp = np.ascontiguousarray(xd).reshape(
            128, ND * 4 * ROWS
        ).astype(ml_dtypes.bfloat16)

        # stationary columns in stream-slot order
        is_win = np.isin(slots, win)
        sgn = np.where(slots * 128 > win[-1] * 128, 1.0, -1.0)
        cs = slots[None, :] * 128 + p128[:, None] - m   # [128, 64]
        wvc = np.zeros((128, 2 * NQ), dtype=np.float32)
        wvc[:, 0::2] = np.where(is_win[None, :], 1.0, cs * sgn[None, :])
        wvc[:, 1::2] = np.where(is_win[None, :], 0.0, sgn[None, :])

        norm = np.sqrt(C * tc.astype(np.float64) ** 2 - 2.0 * tc * s1 + s2)
        sc64 = COEFF / np.maximum(norm, 1e-12) * (tc != 0)
        scs = np.stack([sc64, sc64 * (m - tc.astype(np.float64))])
        in_maps.append({
            "x8": x8,
            "xb": xbp,
            "wv": wvc.astype(ml_dtypes.bfloat16),
            "scs": np.ascontiguousarray(scs.astype(np.float32)),
        })
    return in_maps


def run(input: np.ndarray, target: np.ndarray, trace: bool = False, tmpdir=None):
    nc = _get_nc()
    in_maps = make_in_maps(input, target)
    res = run_bass_kernel_spmd(
        nc, in_maps, list(range(N_CORES)), trace=trace, tmpdir=tmpdir
    )
    total = np.float32(0.0)
    for r in res.results:
        total += np.float32(r["out"].reshape(-1).sum())
    return np.asarray(total, dtype=np.float32), res


def kernel(input: np.ndarray, target: np.ndarray) -> np.ndarray:
    out, _ = run(input, target)
    return out
